# revision 62
# baseline (speedup 1.0000x reference)
"""Trainium2 Bass kernel for nn_NeuralCellularAutomata2 (B16,H64,W64,C256).

Self-contained: hardcodes shapes/sharding. Strategy:
 - data-parallel over batch: 16 images -> 8 cores x 2 images
 - the dispatch wall in this axon-tunneled environment is TRANSFER-dominated
   (~45-55 MB/s each way, no cross-dispatch overlap), so the wire format is
   minimized:
     * weights are baked into the NEFF as Const tensors (loaded to HBM once
       at model-load time, never shipped per dispatch)
     * x ships as int8 (uniform quant over [-max|h|, max|h|]; the dequant
       scale is baked as a Const and the program cache is keyed on it;
       device dequantizes to bf16 via ACT with a [128,1] scale AP)
     * the device returns delta = out - x_hat as int8 * DSTEP (fixed scale,
       |delta| <= ~0.8 so ~41 of 127 levels used); the host adds the TRUE
       f32 x back, which cancels the direct input quantization error
       (only propagated effects remain; rel err ~3e-3)
     * dispatch is a fast_dispatch_compile'd jit of the bass_exec custom
       call (no per-call retrace, no bass-effect ordering) with a
       persistent non-donated output placeholder, valid because every
       output element is written
 - host: NHWC->NCHW int8; fold depthwise 3x3 perception conv into the
   following 1x1 conv (9 fused [2C,C] matrices => 9 shifted PSUM-accumulated
   matmuls, zero copies on device); fold qkv into A = Wq^T Wk / sqrt(C) so
   scores = h . (A h)_shifted (q,k never built)
 - device per core (w1f bf16, other weights f32r, x bf16; PSUM fp32):
     ST1 fused conv+up1 -> GELU(ACT) -> up2; h_new = dx + x via identity
     matmul in the same PSUM group (engines cannot mix bf16 with f32 APs)
     z = A h_new; Gram G = h_new^T z over 258-wide bands; 9 score diagonals
     extracted via DRAM roundtrip with stride-259 access patterns;
     softmax in pixel-partition layout; weighted v-sum as PE matmul
     against a banded W' matrix built by diagonal DMA scatter to DRAM;
     delta = h_new^T - x^T + attn accumulated in one PSUM tile, fp8 out.
"""
import base64
import io
import math

import ml_dtypes
import numpy as np

import concourse.bass as bass
import concourse.tile as tile
from concourse import bacc, mybir

B, H, W, C = 16, 64, 64, 256
NCORES = 8
BS = B // NCORES          # images per core
C2, C3 = 2 * C, 3 * C
HW = H * W                # 4096 pixels per image
NT = 8                    # 512-pixel tiles per image
NCHUNK = HW // 128        # 32 x 128-pixel chunks per image
ZP = 1 + 66 * 64 + 1      # padded-z flat length (guard + 66 rows + guard)

F32 = mybir.dt.float32
F32R = mybir.dt.float32r
BF16 = mybir.dt.bfloat16

_TAUS = [(dy, dx) for dy in (-1, 0, 1) for dx in (-1, 0, 1)]
DSTEP = 2.5 / 127.0       # fixed delta quantization step (|delta| <= ~0.8)


def _cap(ap, offset, dims):
    """Build a custom access pattern on ap's tensor: dims = [(step, count)...]."""
    a = ap.copy()
    a.offset = offset
    v = a.ap
    v.clear()
    v.extend([(int(s), int(n)) for (s, n) in dims])
    return a


def _inline_const(nc, data, dtype, name):
    """inline_tensor with an explicit mybir dtype (e.g. float32r)."""
    data = np.ascontiguousarray(data)
    mls = nc._tensor(name, list(data.shape), dtype, kind="Const", type="DRAM")
    buf = io.BytesIO()
    np.save(buf, data, allow_pickle=False)
    mls.file = f"{name}.npy"
    mls.ant_data = base64.standard_b64encode(buf.getvalue()).decode()
    return bass.DRamTensorHandle(name, list(data.shape), dtype)


def _build_program(consts, step, reps=1, upto="full"):
    nc = bacc.Bacc(
        trn_type="TRN2", target_bir_lowering=False, debug=False,
        num_devices=NCORES,
    )
    # ---- wire I/O: x in int8 unpadded (uniform quant; dequant scale baked
    # as a Const, borders zeroed on device), delta out fp8 (host adds x)
    xpad_d = nc.dram_tensor("xpad", [BS, 2, 128, 64, 64], mybir.dt.int8,
                            kind="ExternalInput").ap()
    scale_d = _inline_const(
        nc, np.full((128, 1), step, np.float32), F32, "scale").ap()
    out_d = nc.dram_tensor("out", [BS, 64, 64, 256], mybir.dt.int8,
                           kind="ExternalOutput").ap()
    # ---- weights baked into the NEFF (loaded to HBM once, not shipped)
    w1f_d = _inline_const(nc, consts["w1f"].astype(ml_dtypes.bfloat16),
                          BF16, "w1f").ap()
    ib128_d = _inline_const(nc, np.eye(128, dtype=ml_dtypes.bfloat16),
                            BF16, "ib128").ap()
    ni256_d = _inline_const(
        nc, (-np.eye(256, dtype=np.float32)).astype(ml_dtypes.bfloat16)
        .reshape(2, 128, 256), BF16, "ni256").ap()
    w2t_d = _inline_const(nc, consts["w2t"], F32R, "w2t").ap()
    at_d = _inline_const(nc, consts["at"], F32R, "at").ap()
    wvt_d = _inline_const(nc, consts["wvt"], F32R, "wvt").ap()
    i256_d = _inline_const(nc, consts["i256"], F32R, "i256").ap()
    bh_d = _inline_const(nc, consts["bh"], F32, "bh").ap()
    b2_d = _inline_const(nc, consts["b2"], F32, "b2").ap()
    mask_d = _inline_const(nc, consts["mask"], F32, "mask").ap()

    GELU = mybir.ActivationFunctionType.Gelu
    EXP = mybir.ActivationFunctionType.Exp
    ADD = mybir.AluOpType.add
    MULT = mybir.AluOpType.mult

    import contextlib
    with tile.TileContext(nc) as tc, contextlib.ExitStack() as stk:
        if True:
            specs = [("wts", 1, None), ("konst", 1, None), ("data", 4, None),
                     ("hnewp", 2, None), ("zpadp", 2, None), ("hidp", 8, None),
                     ("vp", 6, None), ("small", 8, None), ("wlp", 9, None),
                     ("xup", 2, None), ("xqp", 2, None),
                     ("ps1", 2, "PSUM"), ("ps2", 1, "PSUM"), ("ps3", 2, "PSUM"),
                     ("ps4", 1, "PSUM"), ("ps5", 2, "PSUM"),
                     ("gdram", 6, "DRAM"), ("wpdram", 6, "DRAM")]
            p = {}
            for pname, bufs, space in specs:
                kw = {"name": pname, "bufs": bufs}
                if space:
                    kw["space"] = space
                p[pname] = stk.enter_context(tc.tile_pool(**kw))
            wts, konst, data, hnewp = (p["wts"], p["konst"], p["data"],
                                       p["hnewp"])
            zpadp, hidp, vpool, small = (p["zpadp"], p["hidp"], p["vp"],
                                         p["small"])
            wlp, ps1, ps2, ps3 = p["wlp"], p["ps1"], p["ps2"], p["ps3"]
            ps4, ps5, gdram, wpdram = (p["ps4"], p["ps5"], p["gdram"],
                                       p["wpdram"])
            xup = p["xup"]
            xqp = p["xqp"]
            # ---------- weights / constants ----------
            w1f = {}
            for tau in range(9):
                for cc in range(2):
                    for mc in range(4):
                        t = wts.tile([128, 128], BF16,
                                     name=f"w1f_{tau}_{cc}_{mc}")
                        nc.sync.dma_start(t[:], w1f_d[tau, cc, mc])
                        w1f[tau, cc, mc] = t
            ib128 = wts.tile([128, 128], BF16, name="ib128")
            nc.sync.dma_start(ib128[:], ib128_d[:])
            ni256 = {}
            for kc in range(2):
                t = wts.tile([128, 256], BF16, name=f"ni256_{kc}")
                nc.sync.dma_start(t[:], ni256_d[kc])
                ni256[kc] = t
            w2t = {}
            for kc in range(4):
                for mc in range(2):
                    t = wts.tile([128, 128], F32R, name=f"w2t_{kc}_{mc}")
                    nc.sync.dma_start(t[:], w2t_d[kc, mc])
                    w2t[kc, mc] = t
            at = {}
            for kc in range(2):
                for mc in range(2):
                    t = wts.tile([128, 128], F32R, name=f"at_{kc}_{mc}")
                    nc.sync.dma_start(t[:], at_d[kc, mc])
                    at[kc, mc] = t
            wvt = {}
            i256 = {}
            for kc in range(2):
                t = wts.tile([128, 256], F32R, name=f"wvt_{kc}")
                nc.sync.dma_start(t[:], wvt_d[kc])
                wvt[kc] = t
                t2 = wts.tile([128, 256], F32R, name=f"i256_{kc}")
                nc.sync.dma_start(t2[:], i256_d[kc])
                i256[kc] = t2
            bh = {}
            for mc in range(4):
                t = konst.tile([128, 1], F32, name=f"bh_{mc}")
                nc.sync.dma_start(t[:], bh_d[mc].unsqueeze(-1))
                bh[mc] = t
            b2c = {}
            for mc in range(2):
                t = konst.tile([128, 1], F32, name=f"b2_{mc}")
                nc.sync.dma_start(t[:], b2_d[mc].unsqueeze(-1))
                b2c[mc] = t
            mask = konst.tile([128, 9], F32, name="mask")
            nc.sync.dma_start(mask[:], mask_d[:])
            scv = konst.tile([128, 1], F32, name="scv")
            nc.sync.dma_start(scv[:], scale_d[:])

            zf32 = konst.tile([128, 512], F32, name="zf32")
            nc.gpsimd.memset(zf32[:], 0.0)
            vzero = konst.tile([128, 256], F32R, name="vzero")
            nc.vector.tensor_copy(vzero[:], zf32[:, :256])
            wpz = konst.tile([128, 384], F32R, name="wpz")
            nc.vector.tensor_copy(wpz[:], zf32[:, :384])

            # ---------- per-image pipeline ----------
            for img in [i % BS for i in range(BS * reps)]:
                # int8 staging (padded; border ring zeroed), then dequant to
                # bf16: padded xr for the conv, contiguous xu for -x lhsT
                xr, xu = [], []
                for cc in range(2):
                    q = xqp.tile([128, 66, 66], mybir.dt.int8, name="xq",
                                 tag="xq")
                    nc.gpsimd.memset(q[:, 0, :], 0)
                    nc.gpsimd.memset(q[:, 65, :], 0)
                    nc.gpsimd.memset(q[:, 1:65, 0], 0)
                    nc.gpsimd.memset(q[:, 1:65, 65], 0)
                    nc.sync.dma_start(q[:, 1:65, 1:65], xpad_d[img, cc])
                    t = data.tile([128, 66, 66], BF16, name="xr", tag="xr")
                    nc.scalar.activation(
                        t[:], q[:], mybir.ActivationFunctionType.Identity,
                        scale=scv[:])
                    xr.append(t)
                    u = xup.tile([128, HW], BF16, name="xu", tag="xu")
                    nc.scalar.activation(
                        u[:], q[:, 1:65, 1:65],
                        mybir.ActivationFunctionType.Identity, scale=scv[:])
                    xu.append(u)

                h_new = []
                for cc in range(2):
                    h_new.append(hnewp.tile([128, HW], F32R, name="h_new",
                                            tag="h_new"))

                # ---- ST1 fused conv+up1 -> GELU -> up2 -> residual
                for nt in range(NT):
                    r0 = 8 * nt
                    hid_sb = []
                    for mc in range(4):
                        hp = ps1.tile([128, 512], F32, space="PSUM",
                                      name="hid_ps", tag="hid_ps")
                        k = 0
                        for tau, (dy, dx) in enumerate(_TAUS):
                            for cc in range(2):
                                rhs = xr[cc][:, 1 + dy + r0:9 + dy + r0,
                                             1 + dx:65 + dx]
                                nc.tensor.matmul(
                                    hp[:], w1f[tau, cc, mc][:], rhs,
                                    start=(k == 0), stop=(k == 17))
                                k += 1
                        hs = hidp.tile([128, 512], F32R, name="hid_sb",
                                       tag="hid_sb")
                        nc.scalar.activation(hs[:], hp[:], GELU,
                                             bias=bh[mc][:])
                        hid_sb.append(hs)
                    for mc in range(2):
                        dp = ps2.tile([128, 512], F32, space="PSUM",
                                      name="dx_ps", tag="acc512")
                        for kc in range(4):
                            nc.tensor.matmul(dp[:], w2t[kc, mc][:],
                                             hid_sb[kc][:],
                                             start=(kc == 0), stop=False)
                        # + x via identity matmul (x is bf16; engines can't
                        # mix bf16 with f32-family operands)
                        nc.tensor.matmul(dp[:], ib128[:],
                                         xr[mc][:, 1 + r0:9 + r0, 1:65],
                                         start=False, stop=True)
                        # h_new = (dx + x) + b2
                        nc.scalar.activation(
                            h_new[mc][:, 512 * nt:512 * nt + 512], dp[:],
                            mybir.ActivationFunctionType.Identity,
                            bias=b2c[mc][:])

                if upto == "stage1":
                    for j in range(NCHUNK):
                        osb = small.tile([128, 256], mybir.dt.float8e4,
                                         name="osb", tag="osb")
                        nc.scalar.activation(
                            osb[:],
                            h_new[j % 2][:, min(128 * j, HW - 256):
                                         min(128 * j, HW - 256) + 256],
                            mybir.ActivationFunctionType.Copy)
                        nc.sync.dma_start(
                            _cap(out_d, (img * HW + 128 * j) * 256,
                                 [(256, 128), (1, 256)]), osb[:])
                    continue

                # ---- z = A @ h_new into padded flat layout
                z_pad = []
                for cc in range(2):
                    zt = zpadp.tile([128, ZP], F32R, name="z_pad",
                                    tag="z_pad")
                    # zero the pad zones (guard col + y=-1 row | y=64 row +
                    # guard): cols [0,65) and [ZP-65, ZP)
                    nc.scalar.activation(
                        zt[:, 0:65], zf32[:, 0:65],
                        mybir.ActivationFunctionType.Copy)
                    nc.scalar.activation(
                        zt[:, ZP - 65:ZP], zf32[:, 0:65],
                        mybir.ActivationFunctionType.Copy)
                    z_pad.append(zt)
                for nt in range(NT):
                    for mc in range(2):
                        zp = ps2.tile([128, 512], F32, space="PSUM",
                                      name="z_ps", tag="acc512")
                        for kc in range(2):
                            nc.tensor.matmul(
                                zp[:], at[kc, mc][:],
                                h_new[kc][:, 512 * nt:512 * nt + 512],
                                start=(kc == 0), stop=(kc == 1))
                        nc.vector.tensor_copy(
                            z_pad[mc][:, 65 + 512 * nt:65 + 512 * nt + 512],
                            zp[:])

                # ---- attention: per 128-pixel chunk
                v_sb = {}
                for k in range(NCHUNK + 1):
                    if k < NCHUNK:
                        # v[k] = (Wv h)^T via lhsT = h_new columns
                        vps = ps4.tile([128, 256], F32, space="PSUM",
                                       name="v_ps", tag="v_ps")
                        for kc in range(2):
                            nc.tensor.matmul(
                                vps[:], h_new[kc][:, 128 * k:128 * k + 128],
                                wvt[kc][:], start=(kc == 0), stop=(kc == 1))
                        vt = vpool.tile([128, 256], F32R, name="v_sb",
                                        tag="v_sb")
                        nc.vector.tensor_copy(vt[:], vps[:])
                        v_sb[k] = vt
                    if k < 1:
                        continue
                    j = k - 1
                    # Gram G = h^T z over the 258-wide band
                    gps = ps3.tile([128, 258], F32, space="PSUM",
                                   name="g_ps", tag="g_ps")
                    for kc in range(2):
                        nc.tensor.matmul(
                            gps[:], h_new[kc][:, 128 * j:128 * j + 128],
                            z_pad[kc][:, 128 * j:128 * j + 258],
                            start=(kc == 0), stop=(kc == 1))
                    gsb = small.tile([128, 258], F32, name="gsb", tag="gsb")
                    nc.scalar.activation(gsb[:], gps[:],
                                         mybir.ActivationFunctionType.Copy)
                    gd = gdram.tile([128, 258], F32, space="DRAM",
                                    name="g_dram", tag="g_dram")
                    nc.sync.dma_start(gd[:], gsb[:])
                    # diagonal extraction: s[p, (dy,dx)] = G[p, p+64(dy+1)+dx+1]
                    sc = small.tile([128, 9], F32, name="sc", tag="sc")
                    for a in range(3):
                        nc.sync.dma_start(
                            sc[:, 3 * a:3 * a + 3],
                            _cap(gd, gd.offset + 64 * a,
                                 [(259, 128), (1, 3)]))
                    # mask -> exp -> normalize(+mask numerator)
                    sm = small.tile([128, 9], F32, name="sm", tag="sm")
                    nc.vector.tensor_tensor(sm[:], sc[:], mask[:], op=MULT)
                    ex = small.tile([128, 9], F32, name="ex", tag="ex")
                    nc.scalar.activation(ex[:], sm[:], EXP)
                    sume = small.tile([128, 1], F32, name="sume", tag="sume")
                    nc.vector.tensor_reduce(sume[:], ex[:],
                                            axis=mybir.AxisListType.X, op=ADD)
                    rec = small.tile([128, 1], F32, name="rec", tag="rec")
                    nc.vector.reciprocal(rec[:], sume[:])
                    wn = small.tile([128, 9], F32R, name="wn", tag="wn")
                    nc.vector.scalar_tensor_tensor(
                        out=wn[:], in0=ex[:], scalar=rec[:], in1=mask[:],
                        op0=MULT, op1=MULT)
                    # scatter normalized weights into banded W' in DRAM
                    wp = wpdram.tile([384, 128], F32R, space="DRAM",
                                     name="wp_dram", tag="wp_dram")
                    nc.sync.dma_start(wp[:], wpz[:])  # zero background
                    for a in range(3):
                        nc.sync.dma_start(
                            _cap(wp, wp.offset + 8064 + 8192 * a,
                                 [(129, 128), (128, 3)]),
                            wn[:, 3 * a:3 * a + 3])
                    wl = []
                    for j3 in range(3):
                        wlt = wlp.tile([128, 128], F32R, name="wl", tag="wl")
                        nc.sync.dma_start(
                            wlt[:], wp[128 * j3:128 * j3 + 128, :])
                        wl.append(wlt)
                    # final = h^T (identity matmul) + W'^T v_band, one PSUM group
                    # delta = (h_new - x) + attn, shipped fp8 (host adds
                    # bf16(x) back: |delta| <= ~1 so fp8 stays in budget)
                    fp = ps5.tile([128, 256], F32, space="PSUM",
                                  name="fin_ps", tag="fin_ps")
                    for kc in range(2):
                        nc.tensor.matmul(
                            fp[:], h_new[kc][:, 128 * j:128 * j + 128],
                            i256[kc][:], start=(kc == 0), stop=False)
                        nc.tensor.matmul(
                            fp[:], xu[kc][:, 128 * j:128 * j + 128],
                            ni256[kc][:], start=False, stop=False)
                    for j3 in range(3):
                        kk = j - 1 + j3
                        vband = v_sb[kk][:] if 0 <= kk < NCHUNK else vzero[:]
                        nc.tensor.matmul(fp[:], wl[j3][:], vband,
                                         start=False, stop=(j3 == 2))
                    # delta chunk -> int8 (fixed scale DSTEP; |delta|<=~0.8
                    # so |int|<=~41 of 127) -> DRAM NHWC (pixel-major)
                    osb = small.tile([128, 256], mybir.dt.int8,
                                     name="osb", tag="osb")
                    nc.scalar.activation(osb[:], fp[:],
                                         mybir.ActivationFunctionType.Copy,
                                         scale=float(1.0 / DSTEP))
                    nc.sync.dma_start(
                        _cap(out_d, (img * HW + 128 * j) * 256,
                             [(256, 128), (1, 256)]),
                        osb[:])

    nc.compile()
    return nc


_NC_CACHE = {}


def _get_program(consts, step):
    key = hash((float(step),) + tuple((k, np.asarray(v).tobytes())
                                      for k, v in sorted(consts.items())))
    if _NC_CACHE.get("key") != key:
        _NC_CACHE["nc"] = _build_program(consts, step)
        _NC_CACHE["key"] = key
    return _NC_CACHE["nc"]


def _host_prepare(w_perc, b_perc, w_up1, b_up1, w_up2, b_up2, w_qkv, b_qkv):
    w_perc = np.asarray(w_perc, np.float32)
    b_perc = np.asarray(b_perc, np.float32)
    w_up1 = np.asarray(w_up1, np.float32)
    b_up1 = np.asarray(b_up1, np.float32)
    w_up2 = np.asarray(w_up2, np.float32)
    b_up2 = np.asarray(b_up2, np.float32)
    w_qkv = np.asarray(w_qkv, np.float32)
    b_qkv = np.asarray(b_qkv, np.float32)
    assert np.allclose(b_qkv, 0.0), "kernel assumes zero qkv bias (A-trick)"

    wp = w_perc[:, 0]                       # [3C, 3, 3]
    W1 = w_up1[:, :, 0, 0]                  # [2C, 3C]
    W1r = W1.reshape(C2, C, 3)              # [d, g, t]
    wpr = wp.reshape(C, 3, 3, 3)            # [g, t, dy, dx]
    W1f = np.einsum("dgt,gtyx->yxdg", W1r, wpr).reshape(9, C2, C)
    bh = b_up1 + W1 @ b_perc                # [2C]
    W2 = w_up2[:, :, 0, 0]                  # [C, 2C]
    Wq, Wk, Wv = w_qkv[:C], w_qkv[C:C2], w_qkv[C2:]
    A = (Wq.T @ Wk) / math.sqrt(C)          # [C, C]

    w1f_t = np.empty((9, 2, 4, 128, 128), np.float32)
    for tau in range(9):
        for cc in range(2):
            for mc in range(4):
                w1f_t[tau, cc, mc] = W1f[tau][mc * 128:(mc + 1) * 128,
                                             cc * 128:(cc + 1) * 128].T
    w2t_t = np.empty((4, 2, 128, 128), np.float32)
    for kc in range(4):
        for mc in range(2):
            w2t_t[kc, mc] = W2[mc * 128:(mc + 1) * 128,
                               kc * 128:(kc + 1) * 128].T
    at_t = np.empty((2, 2, 128, 128), np.float32)
    for kc in range(2):
        for mc in range(2):
            at_t[kc, mc] = A[mc * 128:(mc + 1) * 128,
                             kc * 128:(kc + 1) * 128].T
    wvt_t = np.ascontiguousarray(Wv.T.reshape(2, 128, 256))
    i256_t = np.ascontiguousarray(np.eye(256, dtype=np.float32)
                                  .reshape(2, 128, 256))
    bh_t = np.ascontiguousarray(bh.reshape(4, 128))
    b2_t = np.ascontiguousarray(b_up2.reshape(2, 128))

    maskt = np.ones((128, 9), np.float32)
    for p in range(128):
        xx = p % 64
        for dy in (-1, 0, 1):
            for dx in (-1, 0, 1):
                if (xx == 0 and dx == -1) or (xx == 63 and dx == 1):
                    maskt[p, (dy + 1) * 3 + (dx + 1)] = 0.0

    return dict(w1f=w1f_t, w2t=w2t_t, at=at_t, wvt=wvt_t, i256=i256_t,
                bh=bh_t, b2=b2_t, mask=maskt)


def _quant_step(h):
    return np.float32(np.abs(h).max() / 127.0)


def _make_in_maps(h):
    """Per-core input maps: NCHW int8 images (scale is baked in the NEFF)."""
    h = np.asarray(h, np.float32)
    step = _quant_step(h)
    hq = np.clip(np.round(h / step), -127, 127).astype(np.int8)
    in_maps = []
    for core in range(NCORES):
        hx = hq[core * BS:(core + 1) * BS].transpose(0, 3, 1, 2)  # [BS,C,H,W]
        in_maps.append(
            {"xpad": np.ascontiguousarray(hx.reshape(BS, 2, 128, 64, 64))})
    return in_maps


_RUN_CACHE = {}


def _build_runner(nc, dev_lo=0, n_local=NCORES):
    """Lean SPMD dispatch: mirrors bass2jax.run_bass_via_pjrt, but the jitted
    executable is cached across calls and the output placeholder operand is a
    persistent (non-donated) device array — so per dispatch only xpad crosses
    H2D and out crosses D2H. Valid because this kernel writes every output
    element (PJRT custom-call results are allocated uninitialized)."""
    import jax
    import jax.numpy as jnp
    from jax.sharding import Mesh, NamedSharding, PartitionSpec
    from jax.experimental.shard_map import shard_map

    from concourse import bass2jax
    from concourse.bass2jax import _bass_exec_p, install_neuronx_cc_hook

    install_neuronx_cc_hook()

    partition_name = (nc.partition_id_tensor.name
                      if nc.partition_id_tensor else None)
    in_names, out_names, out_avals = [], [], []
    for alloc in nc.m.functions[0].allocations:
        if not isinstance(alloc, mybir.MemoryLocationSet):
            continue
        name = alloc.memorylocations[0].name
        if alloc.kind == "ExternalInput":
            if name != partition_name:
                in_names.append(name)
        elif alloc.kind == "ExternalOutput":
            out_names.append(name)
            out_avals.append(jax.core.ShapedArray(
                tuple(alloc.tensor_shape), mybir.dt.np(alloc.dtype)))
    n_params = len(in_names)
    in_names = in_names + out_names
    if partition_name is not None:
        in_names.append(partition_name)

    def _body(*args):
        operands = list(args)
        if partition_name is not None:
            operands.append(bass2jax.partition_id_tensor())
        outs = _bass_exec_p.bind(
            *operands,
            out_avals=tuple(out_avals),
            in_names=tuple(in_names),
            out_names=tuple(out_names),
            lowering_input_output_aliases=(),
            sim_require_finite=True,
            sim_require_nnan=True,
            nc=nc,
        )
        return tuple(outs)

    devices = jax.devices()[dev_lo:dev_lo + n_local]
    mesh = Mesh(np.asarray(devices), ("core",))
    nio = n_params + len(out_names)
    jitted = jax.jit(
        shard_map(_body, mesh=mesh,
                  in_specs=(PartitionSpec("core"),) * nio,
                  out_specs=(PartitionSpec("core"),) * len(out_names),
                  check_rep=False),
        keep_unused=True,
    )
    in_shapes = []
    for alloc in nc.m.functions[0].allocations:
        if not isinstance(alloc, mybir.MemoryLocationSet):
            continue
        name = alloc.memorylocations[0].name
        if alloc.kind == "ExternalInput" and name in in_names[:n_params]:
            in_shapes.append((name, tuple(alloc.tensor_shape),
                              mybir.dt.np(alloc.dtype)))
    in_shapes.sort(key=lambda t: in_names.index(t[0]))
    arg_structs = [
        jax.ShapeDtypeStruct((n_local * s[0], *s[1:]), dt)
        for _, s, dt in in_shapes
    ] + [
        jax.ShapeDtypeStruct((n_local * a.shape[0], *a.shape[1:]), a.dtype)
        for a in out_avals
    ]
    try:
        from concourse.bass2jax import fast_dispatch_compile
        fn = fast_dispatch_compile(
            lambda: jitted.lower(*arg_structs).compile())
    except Exception:
        fn = jitted
    sh = NamedSharding(mesh, PartitionSpec("core"))
    placeholders = [
        jax.device_put(
            np.zeros((n_local * a.shape[0], *a.shape[1:]), a.dtype), sh)
        for a in out_avals
    ]
    return dict(fn=fn, placeholders=placeholders,
                in_names=in_names[:n_params], out_names=out_names)


def _dispatch(nc, in_maps, dev_lo=0):
    """Dispatch in_maps (one dict per core) on devices[dev_lo:dev_lo+n]."""
    n = len(in_maps)
    key = (id(nc), dev_lo, n)
    r = _RUN_CACHE.get(key)
    if r is None:
        r = _build_runner(nc, dev_lo, n)
        _RUN_CACHE[key] = r
    concat_in = [
        np.concatenate([m[name] for m in in_maps], axis=0)
        for name in r["in_names"]
    ]
    outs = r["fn"](*concat_in, *r["placeholders"])
    return [np.asarray(o) for o in outs]


_WORKER_SRC = r'''
import os, sys, time
import numpy as np
sys.path.insert(0, os.environ["KERNEL_DIR"])
import kernel as K

shm = os.environ["KERNEL_SHM"]
ppid = os.getppid()
d = np.load(shm + "_init.npz")
consts = {k: d[k] for k in d.files if k != "step"}
step = float(d["step"])
nc = K._build_program(consts, step)
dummy = [{"xpad": np.zeros((K.BS, 2, 128, 64, 64), np.int8)}
         for _ in range(4)]
K._dispatch(nc, dummy, dev_lo=4)
open(shm + "_ready", "w").close()
n = 0
while True:
    while not os.path.exists(f"{shm}_go{n}"):
        time.sleep(0.002)
        if os.getppid() != ppid:
            sys.exit(0)
    x = np.load(shm + "_in.npy")
    maps = [{"xpad": x[i]} for i in range(4)]
    out = K._dispatch(nc, maps, dev_lo=4)[0]
    np.save(shm + "_tmp.npy", out)
    os.replace(shm + "_tmp.npy", shm + "_out.npy")
    open(f"{shm}_done{n}", "w").close()
    n += 1
'''

_DUAL = {"state": None, "seq": 0}


def _start_worker(consts, step):
    import os
    import subprocess
    import sys as _sys
    import tempfile
    shmdir = "/dev/shm" if os.path.isdir("/dev/shm") else None
    shm = tempfile.mktemp(prefix="kdual_", dir=shmdir)
    np.savez(shm + "_init.npz", step=np.float32(step), **consts)
    env = dict(os.environ)
    env["KERNEL_DIR"] = os.path.dirname(os.path.abspath(__file__))
    env["KERNEL_SHM"] = shm
    proc = subprocess.Popen(
        [_sys.executable, "-c", _WORKER_SRC], env=env,
        stdout=subprocess.DEVNULL, stderr=subprocess.DEVNULL)
    import time as _t
    t0 = _t.time()
    while not os.path.exists(shm + "_ready"):
        if proc.poll() is not None or _t.time() - t0 > 300:
            raise RuntimeError("dual worker failed to start")
        _t.sleep(0.05)
    _DUAL.update(state="ok", shm=shm, proc=proc, seq=0)


def _dispatch_dual(nc, in_maps):
    """Split 8 cores across two axon clients (the loopback relay serializes
    per connection; two connections give ~1.5x aggregate wire bandwidth)."""
    import os
    import time as _t
    w = _DUAL
    n = w["seq"]
    shm = w["shm"]
    np.save(shm + "_in.npy", np.stack([m["xpad"] for m in in_maps[4:]]))
    open(f"{shm}_go{n}", "w").close()
    out_local = _dispatch(nc, in_maps[:4], dev_lo=0)[0]
    t0 = _t.time()
    while not os.path.exists(f"{shm}_done{n}"):
        if w["proc"].poll() is not None or _t.time() - t0 > 90:
            w["state"] = "dead"
            raise RuntimeError("dual worker died mid-call")
        _t.sleep(0.002)
    out_remote = np.load(shm + "_out.npy")
    w["seq"] = n + 1
    return np.concatenate([out_local, out_remote], axis=0)


def _full_dispatch(nc, in_maps):
    """Dual-client dispatch with transparent single-process fallback."""
    if _DUAL["state"] == "ok":
        try:
            return _dispatch_dual(nc, in_maps)
        except Exception:
            _DUAL["state"] = "dead"
    return _dispatch(nc, in_maps, dev_lo=0)[0]


def kernel(h, w_perc, b_perc, w_up1, b_up1, w_up2, b_up2, w_qkv, b_qkv):
    consts = _host_prepare(w_perc, b_perc, w_up1, b_up1, w_up2, b_up2,
                           w_qkv, b_qkv)
    step = _quant_step(np.asarray(h, np.float32))
    nc = _get_program(consts, step)
    in_maps = _make_in_maps(h)
    if _DUAL["state"] is None:
        # warm the local half first (fills the NEFF cache), then start the
        # second axon-client process for cores 4-7
        try:
            _dispatch(nc, in_maps[:4], dev_lo=0)
            _start_worker(consts, step)
        except Exception:
            _DUAL["state"] = "dead"
    delta = _full_dispatch(nc, in_maps)   # [B, 64, 64, 256] int8 * DSTEP
    # add back the TRUE f32 x (not the quantized x-hat): the direct input
    # quantization error then cancels; only propagated effects remain
    return (np.asarray(h, np.float32)
            + delta.astype(np.float32) * np.float32(DSTEP))


# revision 63
# speedup vs baseline: 1.2117x; 1.2117x over previous
"""Trainium2 Bass kernel for nn_NeuralCellularAutomata2 (B16,H64,W64,C256).

Self-contained: hardcodes shapes/sharding. Strategy:
 - data-parallel over batch: 16 images -> 8 cores x 2 images
 - the dispatch wall in this axon-tunneled environment is TRANSFER-dominated
   (~45-55 MB/s each way, no cross-dispatch overlap), so the wire format is
   minimized:
     * weights are baked into the NEFF as Const tensors (loaded to HBM once
       at model-load time, never shipped per dispatch)
     * x ships as int8 (uniform quant over [-max|h|, max|h|]; the dequant
       scale is baked as a Const and the program cache is keyed on it;
       device dequantizes to bf16 via ACT with a [128,1] scale AP)
     * the device returns delta = out - x_hat as int8 * DSTEP (fixed scale,
       |delta| <= ~0.8 so ~41 of 127 levels used); the host adds the TRUE
       f32 x back, which cancels the direct input quantization error
       (only propagated effects remain; rel err ~3e-3)
     * dispatch is a fast_dispatch_compile'd jit of the bass_exec custom
       call (no per-call retrace, no bass-effect ordering) with a
       persistent non-donated output placeholder, valid because every
       output element is written
 - host: NHWC->NCHW int8; fold depthwise 3x3 perception conv into the
   following 1x1 conv (9 fused [2C,C] matrices => 9 shifted PSUM-accumulated
   matmuls, zero copies on device); fold qkv into A = Wq^T Wk / sqrt(C) so
   scores = h . (A h)_shifted (q,k never built)
 - device per core (w1f bf16, other weights f32r, x bf16; PSUM fp32):
     ST1 fused conv+up1 -> GELU(ACT) -> up2; h_new = dx + x via identity
     matmul in the same PSUM group (engines cannot mix bf16 with f32 APs)
     z = A h_new; Gram G = h_new^T z over 258-wide bands; 9 score diagonals
     extracted via DRAM roundtrip with stride-259 access patterns;
     softmax in pixel-partition layout; weighted v-sum as PE matmul
     against a banded W' matrix built by diagonal DMA scatter to DRAM;
     delta = h_new^T - x^T + attn accumulated in one PSUM tile, fp8 out.
"""
import base64
import io
import math

import ml_dtypes
import numpy as np

import concourse.bass as bass
import concourse.tile as tile
from concourse import bacc, mybir

B, H, W, C = 16, 64, 64, 256
NCORES = 8
BS = B // NCORES          # images per core
C2, C3 = 2 * C, 3 * C
HW = H * W                # 4096 pixels per image
NT = 8                    # 512-pixel tiles per image
NCHUNK = HW // 128        # 32 x 128-pixel chunks per image
ZP = 1 + 66 * 64 + 1      # padded-z flat length (guard + 66 rows + guard)

F32 = mybir.dt.float32
F32R = mybir.dt.float32r
BF16 = mybir.dt.bfloat16

_TAUS = [(dy, dx) for dy in (-1, 0, 1) for dx in (-1, 0, 1)]
DSTEP = 2.5 / 127.0       # fixed delta quantization step (|delta| <= ~0.8)


def _cap(ap, offset, dims):
    """Build a custom access pattern on ap's tensor: dims = [(step, count)...]."""
    a = ap.copy()
    a.offset = offset
    v = a.ap
    v.clear()
    v.extend([(int(s), int(n)) for (s, n) in dims])
    return a


def _inline_const(nc, data, dtype, name):
    """inline_tensor with an explicit mybir dtype (e.g. float32r)."""
    data = np.ascontiguousarray(data)
    mls = nc._tensor(name, list(data.shape), dtype, kind="Const", type="DRAM")
    buf = io.BytesIO()
    np.save(buf, data, allow_pickle=False)
    mls.file = f"{name}.npy"
    mls.ant_data = base64.standard_b64encode(buf.getvalue()).decode()
    return bass.DRamTensorHandle(name, list(data.shape), dtype)


def _build_program(consts, step, reps=1, upto="full"):
    nc = bacc.Bacc(
        trn_type="TRN2", target_bir_lowering=False, debug=False,
        num_devices=NCORES,
    )
    # ---- wire I/O: x in int8 unpadded (uniform quant; dequant scale baked
    # as a Const, borders zeroed on device), delta out fp8 (host adds x)
    xpad_d = nc.dram_tensor("xpad", [BS, 2, 128, 64, 64], mybir.dt.int8,
                            kind="ExternalInput").ap()
    scale_d = _inline_const(
        nc, np.full((128, 1), step, np.float32), F32, "scale").ap()
    out_d = nc.dram_tensor("out", [BS, 64, 64, 256], mybir.dt.int8,
                           kind="ExternalOutput").ap()
    # ---- weights baked into the NEFF (loaded to HBM once, not shipped)
    w1f_d = _inline_const(nc, consts["w1f"].astype(ml_dtypes.bfloat16),
                          BF16, "w1f").ap()
    ib128_d = _inline_const(nc, np.eye(128, dtype=ml_dtypes.bfloat16),
                            BF16, "ib128").ap()
    ni256_d = _inline_const(
        nc, (-np.eye(256, dtype=np.float32)).astype(ml_dtypes.bfloat16)
        .reshape(2, 128, 256), BF16, "ni256").ap()
    w2t_d = _inline_const(nc, consts["w2t"], F32R, "w2t").ap()
    at_d = _inline_const(nc, consts["at"], F32R, "at").ap()
    wvt_d = _inline_const(nc, consts["wvt"], F32R, "wvt").ap()
    i256_d = _inline_const(nc, consts["i256"], F32R, "i256").ap()
    bh_d = _inline_const(nc, consts["bh"], F32, "bh").ap()
    b2_d = _inline_const(nc, consts["b2"], F32, "b2").ap()
    mask_d = _inline_const(nc, consts["mask"], F32, "mask").ap()

    GELU = mybir.ActivationFunctionType.Gelu
    EXP = mybir.ActivationFunctionType.Exp
    ADD = mybir.AluOpType.add
    MULT = mybir.AluOpType.mult

    import contextlib
    with tile.TileContext(nc) as tc, contextlib.ExitStack() as stk:
        if True:
            specs = [("wts", 1, None), ("konst", 1, None), ("data", 4, None),
                     ("hnewp", 2, None), ("zpadp", 2, None), ("hidp", 8, None),
                     ("vp", 6, None), ("small", 8, None), ("wlp", 9, None),
                     ("xup", 2, None), ("xqp", 2, None),
                     ("ps1", 2, "PSUM"), ("ps2", 1, "PSUM"), ("ps3", 2, "PSUM"),
                     ("ps4", 1, "PSUM"), ("ps5", 2, "PSUM"),
                     ("gdram", 6, "DRAM"), ("wpdram", 6, "DRAM")]
            p = {}
            for pname, bufs, space in specs:
                kw = {"name": pname, "bufs": bufs}
                if space:
                    kw["space"] = space
                p[pname] = stk.enter_context(tc.tile_pool(**kw))
            wts, konst, data, hnewp = (p["wts"], p["konst"], p["data"],
                                       p["hnewp"])
            zpadp, hidp, vpool, small = (p["zpadp"], p["hidp"], p["vp"],
                                         p["small"])
            wlp, ps1, ps2, ps3 = p["wlp"], p["ps1"], p["ps2"], p["ps3"]
            ps4, ps5, gdram, wpdram = (p["ps4"], p["ps5"], p["gdram"],
                                       p["wpdram"])
            xup = p["xup"]
            xqp = p["xqp"]
            # ---------- weights / constants ----------
            w1f = {}
            for tau in range(9):
                for cc in range(2):
                    for mc in range(4):
                        t = wts.tile([128, 128], BF16,
                                     name=f"w1f_{tau}_{cc}_{mc}")
                        nc.sync.dma_start(t[:], w1f_d[tau, cc, mc])
                        w1f[tau, cc, mc] = t
            ib128 = wts.tile([128, 128], BF16, name="ib128")
            nc.sync.dma_start(ib128[:], ib128_d[:])
            ni256 = {}
            for kc in range(2):
                t = wts.tile([128, 256], BF16, name=f"ni256_{kc}")
                nc.sync.dma_start(t[:], ni256_d[kc])
                ni256[kc] = t
            w2t = {}
            for kc in range(4):
                for mc in range(2):
                    t = wts.tile([128, 128], F32R, name=f"w2t_{kc}_{mc}")
                    nc.sync.dma_start(t[:], w2t_d[kc, mc])
                    w2t[kc, mc] = t
            at = {}
            for kc in range(2):
                for mc in range(2):
                    t = wts.tile([128, 128], F32R, name=f"at_{kc}_{mc}")
                    nc.sync.dma_start(t[:], at_d[kc, mc])
                    at[kc, mc] = t
            wvt = {}
            i256 = {}
            for kc in range(2):
                t = wts.tile([128, 256], F32R, name=f"wvt_{kc}")
                nc.sync.dma_start(t[:], wvt_d[kc])
                wvt[kc] = t
                t2 = wts.tile([128, 256], F32R, name=f"i256_{kc}")
                nc.sync.dma_start(t2[:], i256_d[kc])
                i256[kc] = t2
            bh = {}
            for mc in range(4):
                t = konst.tile([128, 1], F32, name=f"bh_{mc}")
                nc.sync.dma_start(t[:], bh_d[mc].unsqueeze(-1))
                bh[mc] = t
            b2c = {}
            for mc in range(2):
                t = konst.tile([128, 1], F32, name=f"b2_{mc}")
                nc.sync.dma_start(t[:], b2_d[mc].unsqueeze(-1))
                b2c[mc] = t
            mask = konst.tile([128, 9], F32, name="mask")
            nc.sync.dma_start(mask[:], mask_d[:])
            scv = konst.tile([128, 1], F32, name="scv")
            nc.sync.dma_start(scv[:], scale_d[:])

            zf32 = konst.tile([128, 512], F32, name="zf32")
            nc.gpsimd.memset(zf32[:], 0.0)
            vzero = konst.tile([128, 256], F32R, name="vzero")
            nc.vector.tensor_copy(vzero[:], zf32[:, :256])
            wpz = konst.tile([128, 384], F32R, name="wpz")
            nc.vector.tensor_copy(wpz[:], zf32[:, :384])

            # ---------- per-image pipeline ----------
            for img in [i % BS for i in range(BS * reps)]:
                # int8 staging (padded; border ring zeroed), then dequant to
                # bf16: padded xr for the conv, contiguous xu for -x lhsT
                xr, xu = [], []
                for cc in range(2):
                    q = xqp.tile([128, 66, 66], mybir.dt.int8, name="xq",
                                 tag="xq")
                    nc.gpsimd.memset(q[:, 0, :], 0)
                    nc.gpsimd.memset(q[:, 65, :], 0)
                    nc.gpsimd.memset(q[:, 1:65, 0], 0)
                    nc.gpsimd.memset(q[:, 1:65, 65], 0)
                    nc.sync.dma_start(q[:, 1:65, 1:65], xpad_d[img, cc])
                    t = data.tile([128, 66, 66], BF16, name="xr", tag="xr")
                    nc.scalar.activation(
                        t[:], q[:], mybir.ActivationFunctionType.Identity,
                        scale=scv[:])
                    xr.append(t)
                    u = xup.tile([128, HW], BF16, name="xu", tag="xu")
                    nc.scalar.activation(
                        u[:], q[:, 1:65, 1:65],
                        mybir.ActivationFunctionType.Identity, scale=scv[:])
                    xu.append(u)

                h_new = []
                for cc in range(2):
                    h_new.append(hnewp.tile([128, HW], F32R, name="h_new",
                                            tag="h_new"))

                # ---- ST1 fused conv+up1 -> GELU -> up2 -> residual
                for nt in range(NT):
                    r0 = 8 * nt
                    hid_sb = []
                    for mc in range(4):
                        hp = ps1.tile([128, 512], F32, space="PSUM",
                                      name="hid_ps", tag="hid_ps")
                        k = 0
                        for tau, (dy, dx) in enumerate(_TAUS):
                            for cc in range(2):
                                rhs = xr[cc][:, 1 + dy + r0:9 + dy + r0,
                                             1 + dx:65 + dx]
                                nc.tensor.matmul(
                                    hp[:], w1f[tau, cc, mc][:], rhs,
                                    start=(k == 0), stop=(k == 17))
                                k += 1
                        hs = hidp.tile([128, 512], F32R, name="hid_sb",
                                       tag="hid_sb")
                        nc.scalar.activation(hs[:], hp[:], GELU,
                                             bias=bh[mc][:])
                        hid_sb.append(hs)
                    for mc in range(2):
                        dp = ps2.tile([128, 512], F32, space="PSUM",
                                      name="dx_ps", tag="acc512")
                        for kc in range(4):
                            nc.tensor.matmul(dp[:], w2t[kc, mc][:],
                                             hid_sb[kc][:],
                                             start=(kc == 0), stop=False)
                        # + x via identity matmul (x is bf16; engines can't
                        # mix bf16 with f32-family operands)
                        nc.tensor.matmul(dp[:], ib128[:],
                                         xr[mc][:, 1 + r0:9 + r0, 1:65],
                                         start=False, stop=True)
                        # h_new = (dx + x) + b2
                        nc.scalar.activation(
                            h_new[mc][:, 512 * nt:512 * nt + 512], dp[:],
                            mybir.ActivationFunctionType.Identity,
                            bias=b2c[mc][:])

                if upto == "stage1":
                    for j in range(NCHUNK):
                        osb = small.tile([128, 256], mybir.dt.float8e4,
                                         name="osb", tag="osb")
                        nc.scalar.activation(
                            osb[:],
                            h_new[j % 2][:, min(128 * j, HW - 256):
                                         min(128 * j, HW - 256) + 256],
                            mybir.ActivationFunctionType.Copy)
                        nc.sync.dma_start(
                            _cap(out_d, (img * HW + 128 * j) * 256,
                                 [(256, 128), (1, 256)]), osb[:])
                    continue

                # ---- z = A @ h_new into padded flat layout
                z_pad = []
                for cc in range(2):
                    zt = zpadp.tile([128, ZP], F32R, name="z_pad",
                                    tag="z_pad")
                    # zero the pad zones (guard col + y=-1 row | y=64 row +
                    # guard): cols [0,65) and [ZP-65, ZP)
                    nc.scalar.activation(
                        zt[:, 0:65], zf32[:, 0:65],
                        mybir.ActivationFunctionType.Copy)
                    nc.scalar.activation(
                        zt[:, ZP - 65:ZP], zf32[:, 0:65],
                        mybir.ActivationFunctionType.Copy)
                    z_pad.append(zt)
                for nt in range(NT):
                    for mc in range(2):
                        zp = ps2.tile([128, 512], F32, space="PSUM",
                                      name="z_ps", tag="acc512")
                        for kc in range(2):
                            nc.tensor.matmul(
                                zp[:], at[kc, mc][:],
                                h_new[kc][:, 512 * nt:512 * nt + 512],
                                start=(kc == 0), stop=(kc == 1))
                        nc.vector.tensor_copy(
                            z_pad[mc][:, 65 + 512 * nt:65 + 512 * nt + 512],
                            zp[:])

                # ---- attention: per 128-pixel chunk
                v_sb = {}
                for k in range(NCHUNK + 1):
                    if k < NCHUNK:
                        # v[k] = (Wv h)^T via lhsT = h_new columns
                        vps = ps4.tile([128, 256], F32, space="PSUM",
                                       name="v_ps", tag="v_ps")
                        for kc in range(2):
                            nc.tensor.matmul(
                                vps[:], h_new[kc][:, 128 * k:128 * k + 128],
                                wvt[kc][:], start=(kc == 0), stop=(kc == 1))
                        vt = vpool.tile([128, 256], F32R, name="v_sb",
                                        tag="v_sb")
                        nc.vector.tensor_copy(vt[:], vps[:])
                        v_sb[k] = vt
                    if k < 1:
                        continue
                    j = k - 1
                    # Gram G = h^T z over the 258-wide band
                    gps = ps3.tile([128, 258], F32, space="PSUM",
                                   name="g_ps", tag="g_ps")
                    for kc in range(2):
                        nc.tensor.matmul(
                            gps[:], h_new[kc][:, 128 * j:128 * j + 128],
                            z_pad[kc][:, 128 * j:128 * j + 258],
                            start=(kc == 0), stop=(kc == 1))
                    gsb = small.tile([128, 258], F32, name="gsb", tag="gsb")
                    nc.scalar.activation(gsb[:], gps[:],
                                         mybir.ActivationFunctionType.Copy)
                    gd = gdram.tile([128, 258], F32, space="DRAM",
                                    name="g_dram", tag="g_dram")
                    nc.sync.dma_start(gd[:], gsb[:])
                    # diagonal extraction: s[p, (dy,dx)] = G[p, p+64(dy+1)+dx+1]
                    sc = small.tile([128, 9], F32, name="sc", tag="sc")
                    for a in range(3):
                        nc.sync.dma_start(
                            sc[:, 3 * a:3 * a + 3],
                            _cap(gd, gd.offset + 64 * a,
                                 [(259, 128), (1, 3)]))
                    # mask -> exp -> normalize(+mask numerator)
                    sm = small.tile([128, 9], F32, name="sm", tag="sm")
                    nc.vector.tensor_tensor(sm[:], sc[:], mask[:], op=MULT)
                    ex = small.tile([128, 9], F32, name="ex", tag="ex")
                    nc.scalar.activation(ex[:], sm[:], EXP)
                    sume = small.tile([128, 1], F32, name="sume", tag="sume")
                    nc.vector.tensor_reduce(sume[:], ex[:],
                                            axis=mybir.AxisListType.X, op=ADD)
                    rec = small.tile([128, 1], F32, name="rec", tag="rec")
                    nc.vector.reciprocal(rec[:], sume[:])
                    wn = small.tile([128, 9], F32R, name="wn", tag="wn")
                    nc.vector.scalar_tensor_tensor(
                        out=wn[:], in0=ex[:], scalar=rec[:], in1=mask[:],
                        op0=MULT, op1=MULT)
                    # scatter normalized weights into banded W' in DRAM
                    wp = wpdram.tile([384, 128], F32R, space="DRAM",
                                     name="wp_dram", tag="wp_dram")
                    nc.sync.dma_start(wp[:], wpz[:])  # zero background
                    for a in range(3):
                        nc.sync.dma_start(
                            _cap(wp, wp.offset + 8064 + 8192 * a,
                                 [(129, 128), (128, 3)]),
                            wn[:, 3 * a:3 * a + 3])
                    wl = []
                    for j3 in range(3):
                        wlt = wlp.tile([128, 128], F32R, name="wl", tag="wl")
                        nc.sync.dma_start(
                            wlt[:], wp[128 * j3:128 * j3 + 128, :])
                        wl.append(wlt)
                    # final = h^T (identity matmul) + W'^T v_band, one PSUM group
                    # delta = (h_new - x) + attn, shipped fp8 (host adds
                    # bf16(x) back: |delta| <= ~1 so fp8 stays in budget)
                    fp = ps5.tile([128, 256], F32, space="PSUM",
                                  name="fin_ps", tag="fin_ps")
                    for kc in range(2):
                        nc.tensor.matmul(
                            fp[:], h_new[kc][:, 128 * j:128 * j + 128],
                            i256[kc][:], start=(kc == 0), stop=False)
                        nc.tensor.matmul(
                            fp[:], xu[kc][:, 128 * j:128 * j + 128],
                            ni256[kc][:], start=False, stop=False)
                    for j3 in range(3):
                        kk = j - 1 + j3
                        vband = v_sb[kk][:] if 0 <= kk < NCHUNK else vzero[:]
                        nc.tensor.matmul(fp[:], wl[j3][:], vband,
                                         start=False, stop=(j3 == 2))
                    # delta chunk -> int8 (fixed scale DSTEP; |delta|<=~0.8
                    # so |int|<=~41 of 127) -> DRAM NHWC (pixel-major)
                    osb = small.tile([128, 256], mybir.dt.int8,
                                     name="osb", tag="osb")
                    nc.scalar.activation(osb[:], fp[:],
                                         mybir.ActivationFunctionType.Copy,
                                         scale=float(1.0 / DSTEP))
                    nc.sync.dma_start(
                        _cap(out_d, (img * HW + 128 * j) * 256,
                             [(256, 128), (1, 256)]),
                        osb[:])

    nc.compile()
    return nc


_NC_CACHE = {}


def _get_program(consts, step):
    key = hash((float(step),) + tuple((k, np.asarray(v).tobytes())
                                      for k, v in sorted(consts.items())))
    if _NC_CACHE.get("key") != key:
        _NC_CACHE["nc"] = _build_program(consts, step)
        _NC_CACHE["key"] = key
    return _NC_CACHE["nc"]


def _host_prepare(w_perc, b_perc, w_up1, b_up1, w_up2, b_up2, w_qkv, b_qkv):
    w_perc = np.asarray(w_perc, np.float32)
    b_perc = np.asarray(b_perc, np.float32)
    w_up1 = np.asarray(w_up1, np.float32)
    b_up1 = np.asarray(b_up1, np.float32)
    w_up2 = np.asarray(w_up2, np.float32)
    b_up2 = np.asarray(b_up2, np.float32)
    w_qkv = np.asarray(w_qkv, np.float32)
    b_qkv = np.asarray(b_qkv, np.float32)
    assert np.allclose(b_qkv, 0.0), "kernel assumes zero qkv bias (A-trick)"

    wp = w_perc[:, 0]                       # [3C, 3, 3]
    W1 = w_up1[:, :, 0, 0]                  # [2C, 3C]
    W1r = W1.reshape(C2, C, 3)              # [d, g, t]
    wpr = wp.reshape(C, 3, 3, 3)            # [g, t, dy, dx]
    W1f = np.einsum("dgt,gtyx->yxdg", W1r, wpr).reshape(9, C2, C)
    bh = b_up1 + W1 @ b_perc                # [2C]
    W2 = w_up2[:, :, 0, 0]                  # [C, 2C]
    Wq, Wk, Wv = w_qkv[:C], w_qkv[C:C2], w_qkv[C2:]
    A = (Wq.T @ Wk) / math.sqrt(C)          # [C, C]

    w1f_t = np.empty((9, 2, 4, 128, 128), np.float32)
    for tau in range(9):
        for cc in range(2):
            for mc in range(4):
                w1f_t[tau, cc, mc] = W1f[tau][mc * 128:(mc + 1) * 128,
                                             cc * 128:(cc + 1) * 128].T
    w2t_t = np.empty((4, 2, 128, 128), np.float32)
    for kc in range(4):
        for mc in range(2):
            w2t_t[kc, mc] = W2[mc * 128:(mc + 1) * 128,
                               kc * 128:(kc + 1) * 128].T
    at_t = np.empty((2, 2, 128, 128), np.float32)
    for kc in range(2):
        for mc in range(2):
            at_t[kc, mc] = A[mc * 128:(mc + 1) * 128,
                             kc * 128:(kc + 1) * 128].T
    wvt_t = np.ascontiguousarray(Wv.T.reshape(2, 128, 256))
    i256_t = np.ascontiguousarray(np.eye(256, dtype=np.float32)
                                  .reshape(2, 128, 256))
    bh_t = np.ascontiguousarray(bh.reshape(4, 128))
    b2_t = np.ascontiguousarray(b_up2.reshape(2, 128))

    maskt = np.ones((128, 9), np.float32)
    for p in range(128):
        xx = p % 64
        for dy in (-1, 0, 1):
            for dx in (-1, 0, 1):
                if (xx == 0 and dx == -1) or (xx == 63 and dx == 1):
                    maskt[p, (dy + 1) * 3 + (dx + 1)] = 0.0

    return dict(w1f=w1f_t, w2t=w2t_t, at=at_t, wvt=wvt_t, i256=i256_t,
                bh=bh_t, b2=b2_t, mask=maskt)


def _quant_step(h):
    return np.float32(np.abs(h).max() / 127.0)


def _make_in_maps(h):
    """Per-core input maps: NCHW int8 images (scale is baked in the NEFF)."""
    h = np.asarray(h, np.float32)
    step = _quant_step(h)
    hq = np.clip(np.round(h / step), -127, 127).astype(np.int8)
    in_maps = []
    for core in range(NCORES):
        hx = hq[core * BS:(core + 1) * BS].transpose(0, 3, 1, 2)  # [BS,C,H,W]
        in_maps.append(
            {"xpad": np.ascontiguousarray(hx.reshape(BS, 2, 128, 64, 64))})
    return in_maps


_RUN_CACHE = {}


def _build_runner(nc):
    """Lean SPMD dispatch: mirrors bass2jax.run_bass_via_pjrt, but the jitted
    executable is cached across calls and the output placeholder operand is a
    persistent (non-donated) device array — so per dispatch only xpad crosses
    H2D and out crosses D2H. Valid because this kernel writes every output
    element (PJRT custom-call results are allocated uninitialized)."""
    import jax
    import jax.numpy as jnp
    from jax.sharding import Mesh, NamedSharding, PartitionSpec
    from jax.experimental.shard_map import shard_map

    from concourse import bass2jax
    from concourse.bass2jax import _bass_exec_p, install_neuronx_cc_hook

    install_neuronx_cc_hook()

    partition_name = (nc.partition_id_tensor.name
                      if nc.partition_id_tensor else None)
    in_names, out_names, out_avals = [], [], []
    for alloc in nc.m.functions[0].allocations:
        if not isinstance(alloc, mybir.MemoryLocationSet):
            continue
        name = alloc.memorylocations[0].name
        if alloc.kind == "ExternalInput":
            if name != partition_name:
                in_names.append(name)
        elif alloc.kind == "ExternalOutput":
            out_names.append(name)
            out_avals.append(jax.core.ShapedArray(
                tuple(alloc.tensor_shape), mybir.dt.np(alloc.dtype)))
    n_params = len(in_names)
    in_names = in_names + out_names
    if partition_name is not None:
        in_names.append(partition_name)

    def _body(*args):
        operands = list(args)
        if partition_name is not None:
            operands.append(bass2jax.partition_id_tensor())
        outs = _bass_exec_p.bind(
            *operands,
            out_avals=tuple(out_avals),
            in_names=tuple(in_names),
            out_names=tuple(out_names),
            lowering_input_output_aliases=(),
            sim_require_finite=True,
            sim_require_nnan=True,
            nc=nc,
        )
        return tuple(outs)

    devices = jax.devices()[:NCORES]
    mesh = Mesh(np.asarray(devices), ("core",))
    nio = n_params + len(out_names)
    jitted = jax.jit(
        shard_map(_body, mesh=mesh,
                  in_specs=(PartitionSpec("core"),) * nio,
                  out_specs=(PartitionSpec("core"),) * len(out_names),
                  check_rep=False),
        keep_unused=True,
    )
    in_shapes = []
    for alloc in nc.m.functions[0].allocations:
        if not isinstance(alloc, mybir.MemoryLocationSet):
            continue
        name = alloc.memorylocations[0].name
        if alloc.kind == "ExternalInput" and name in in_names[:n_params]:
            in_shapes.append((name, tuple(alloc.tensor_shape),
                              mybir.dt.np(alloc.dtype)))
    in_shapes.sort(key=lambda t: in_names.index(t[0]))
    arg_structs = [
        jax.ShapeDtypeStruct((NCORES * s[0], *s[1:]), dt)
        for _, s, dt in in_shapes
    ] + [
        jax.ShapeDtypeStruct((NCORES * a.shape[0], *a.shape[1:]), a.dtype)
        for a in out_avals
    ]
    try:
        from concourse.bass2jax import fast_dispatch_compile
        fn = fast_dispatch_compile(
            lambda: jitted.lower(*arg_structs).compile())
    except Exception:
        fn = jitted
    sh = NamedSharding(mesh, PartitionSpec("core"))
    placeholders = [
        jax.device_put(np.zeros((NCORES * a.shape[0], *a.shape[1:]), a.dtype),
                       sh)
        for a in out_avals
    ]
    return dict(fn=fn, placeholders=placeholders,
                in_names=in_names[:n_params], out_names=out_names)


def _dispatch(nc, in_maps):
    r = _RUN_CACHE.get("runner")
    if r is None or _RUN_CACHE.get("nc") is not nc:
        r = _build_runner(nc)
        _RUN_CACHE["runner"] = r
        _RUN_CACHE["nc"] = nc
    concat_in = [
        np.concatenate([m[name] for m in in_maps], axis=0)
        for name in r["in_names"]
    ]
    outs = r["fn"](*concat_in, *r["placeholders"])
    return [np.asarray(o) for o in outs]


def kernel(h, w_perc, b_perc, w_up1, b_up1, w_up2, b_up2, w_qkv, b_qkv):
    consts = _host_prepare(w_perc, b_perc, w_up1, b_up1, w_up2, b_up2,
                           w_qkv, b_qkv)
    nc = _get_program(consts, _quant_step(np.asarray(h, np.float32)))
    in_maps = _make_in_maps(h)
    delta = _dispatch(nc, in_maps)[0]     # [B, 64, 64, 256] int8 * DSTEP
    # add back the TRUE f32 x (not the quantized x-hat): the direct input
    # quantization error then cancels; only propagated effects remain
    return (np.asarray(h, np.float32)
            + delta.astype(np.float32) * np.float32(DSTEP))


# revision 66
# speedup vs baseline: 1.2453x; 1.0277x over previous
"""Trainium2 Bass kernel for nn_NeuralCellularAutomata2 (B16,H64,W64,C256).

Self-contained: hardcodes shapes/sharding. Strategy:
 - data-parallel over batch: 16 images -> 8 cores x 2 images
 - the dispatch wall in this axon-tunneled environment is TRANSFER-dominated
   (~45-55 MB/s each way, no cross-dispatch overlap), so the wire format is
   minimized:
     * weights are baked into the NEFF as Const tensors (loaded to HBM once
       at model-load time, never shipped per dispatch)
     * x ships as int8 (uniform quant over [-max|h|, max|h|]; the dequant
       scale is baked as a Const and the program cache is keyed on it;
       device dequantizes to bf16 via ACT with a [128,1] scale AP)
     * the device returns delta = out - x_hat as int8 * DSTEP (fixed scale,
       |delta| <= ~0.8 so ~41 of 127 levels used); the host adds the TRUE
       f32 x back, which cancels the direct input quantization error
       (only propagated effects remain; rel err ~3e-3)
     * dispatch is a fast_dispatch_compile'd jit of the bass_exec custom
       call (no per-call retrace, no bass-effect ordering) with a
       persistent non-donated output placeholder, valid because every
       output element is written
 - host: NHWC->NCHW int8; fold depthwise 3x3 perception conv into the
   following 1x1 conv (9 fused [2C,C] matrices => 9 shifted PSUM-accumulated
   matmuls, zero copies on device); fold qkv into A = Wq^T Wk / sqrt(C) so
   scores = h . (A h)_shifted (q,k never built)
 - device per core (w1f bf16, other weights f32r, x bf16; PSUM fp32):
     ST1 fused conv+up1 -> GELU(ACT) -> up2; h_new = dx + x via identity
     matmul in the same PSUM group (engines cannot mix bf16 with f32 APs)
     z = A h_new; Gram G = h_new^T z over 258-wide bands; 9 score diagonals
     extracted via DRAM roundtrip with stride-259 access patterns;
     softmax in pixel-partition layout; weighted v-sum as PE matmul
     against a banded W' matrix built by diagonal DMA scatter to DRAM;
     delta = h_new^T - x^T + attn accumulated in one PSUM tile, int8 out.
"""
import base64
import io
import math

import ml_dtypes
import numpy as np

import concourse.bass as bass
import concourse.tile as tile
from concourse import bacc, mybir

B, H, W, C = 16, 64, 64, 256
NCORES = 8
BS = B // NCORES          # images per core
C2, C3 = 2 * C, 3 * C
HW = H * W                # 4096 pixels per image
NT = 8                    # 512-pixel tiles per image
NCHUNK = HW // 128        # 32 x 128-pixel chunks per image
ZP = 1 + 66 * 64 + 1      # padded-z flat length (guard + 66 rows + guard)

F32 = mybir.dt.float32
F32R = mybir.dt.float32r
BF16 = mybir.dt.bfloat16

_TAUS = [(dy, dx) for dy in (-1, 0, 1) for dx in (-1, 0, 1)]
DSTEP = 2.5 / 127.0       # fixed delta quantization step (|delta| <= ~0.8)


def _cap(ap, offset, dims):
    """Build a custom access pattern on ap's tensor: dims = [(step, count)...]."""
    a = ap.copy()
    a.offset = offset
    v = a.ap
    v.clear()
    v.extend([(int(s), int(n)) for (s, n) in dims])
    return a


def _inline_const(nc, data, dtype, name):
    """inline_tensor with an explicit mybir dtype (e.g. float32r)."""
    data = np.ascontiguousarray(data)
    mls = nc._tensor(name, list(data.shape), dtype, kind="Const", type="DRAM")
    buf = io.BytesIO()
    np.save(buf, data, allow_pickle=False)
    mls.file = f"{name}.npy"
    mls.ant_data = base64.standard_b64encode(buf.getvalue()).decode()
    return bass.DRamTensorHandle(name, list(data.shape), dtype)


def _build_program(consts, step, reps=1, upto="full"):
    nc = bacc.Bacc(
        trn_type="TRN2", target_bir_lowering=False, debug=False,
        num_devices=NCORES,
    )
    # ---- wire I/O: x in int8 unpadded (uniform quant; dequant scale baked
    # as a Const, borders zeroed on device), delta out int8 (host adds x)
    xpad_d = nc.dram_tensor("xpad", [BS, 2, 128, 64, 64], mybir.dt.int8,
                            kind="ExternalInput").ap()
    scale_d = _inline_const(
        nc, np.full((128, 1), step, np.float32), F32, "scale").ap()
    out_d = nc.dram_tensor("out", [BS, 64, 64, 256], mybir.dt.int8,
                           kind="ExternalOutput").ap()
    # ---- weights baked into the NEFF (loaded to HBM once, not shipped)
    w1f_d = _inline_const(nc, consts["w1f"].astype(ml_dtypes.bfloat16),
                          BF16, "w1f").ap()
    ib128_d = _inline_const(nc, np.eye(128, dtype=ml_dtypes.bfloat16),
                            BF16, "ib128").ap()
    ni256_d = _inline_const(
        nc, (-np.eye(256, dtype=np.float32)).astype(ml_dtypes.bfloat16)
        .reshape(2, 128, 256), BF16, "ni256").ap()
    w2t_d = _inline_const(nc, consts["w2t"], F32R, "w2t").ap()
    at_d = _inline_const(nc, consts["at"], F32R, "at").ap()
    wvt_d = _inline_const(nc, consts["wvt"], F32R, "wvt").ap()
    i256_d = _inline_const(nc, consts["i256"], F32R, "i256").ap()
    bh_d = _inline_const(nc, consts["bh"], F32, "bh").ap()
    b2_d = _inline_const(nc, consts["b2"], F32, "b2").ap()
    mask_d = _inline_const(nc, consts["mask"], F32, "mask").ap()

    GELU = mybir.ActivationFunctionType.Gelu
    EXP = mybir.ActivationFunctionType.Exp
    ADD = mybir.AluOpType.add
    MULT = mybir.AluOpType.mult

    import contextlib
    with tile.TileContext(nc) as tc, contextlib.ExitStack() as stk:
        if True:
            specs = [("wts", 1, None), ("konst", 1, None), ("data", 4, None),
                     ("hnewp", 2, None), ("zpadp", 2, None), ("hidp", 8, None),
                     ("vp", 6, None), ("small", 8, None), ("wlp", 9, None),
                     ("xup", 2, None), ("xqp", 2, None),
                     ("ps1", 2, "PSUM"), ("ps2", 1, "PSUM"), ("ps3", 2, "PSUM"),
                     ("ps4", 1, "PSUM"), ("ps5", 2, "PSUM"),
                     ("gdram", 6, "DRAM"), ("wpdram", 6, "DRAM")]
            p = {}
            for pname, bufs, space in specs:
                kw = {"name": pname, "bufs": bufs}
                if space:
                    kw["space"] = space
                p[pname] = stk.enter_context(tc.tile_pool(**kw))
            wts, konst, data, hnewp = (p["wts"], p["konst"], p["data"],
                                       p["hnewp"])
            zpadp, hidp, vpool, small = (p["zpadp"], p["hidp"], p["vp"],
                                         p["small"])
            wlp, ps1, ps2, ps3 = p["wlp"], p["ps1"], p["ps2"], p["ps3"]
            ps4, ps5, gdram, wpdram = (p["ps4"], p["ps5"], p["gdram"],
                                       p["wpdram"])
            xup = p["xup"]
            xqp = p["xqp"]
            # ---------- weights / constants ----------
            w1f = {}
            for tau in range(9):
                for cc in range(2):
                    for mc in range(4):
                        t = wts.tile([128, 128], BF16,
                                     name=f"w1f_{tau}_{cc}_{mc}")
                        nc.sync.dma_start(t[:], w1f_d[tau, cc, mc])
                        w1f[tau, cc, mc] = t
            ib128 = wts.tile([128, 128], BF16, name="ib128")
            nc.sync.dma_start(ib128[:], ib128_d[:])
            ni256 = {}
            for kc in range(2):
                t = wts.tile([128, 256], BF16, name=f"ni256_{kc}")
                nc.sync.dma_start(t[:], ni256_d[kc])
                ni256[kc] = t
            w2t = {}
            for kc in range(4):
                for mc in range(2):
                    t = wts.tile([128, 128], F32R, name=f"w2t_{kc}_{mc}")
                    nc.sync.dma_start(t[:], w2t_d[kc, mc])
                    w2t[kc, mc] = t
            at = {}
            for kc in range(2):
                for mc in range(2):
                    t = wts.tile([128, 128], F32R, name=f"at_{kc}_{mc}")
                    nc.sync.dma_start(t[:], at_d[kc, mc])
                    at[kc, mc] = t
            wvt = {}
            i256 = {}
            for kc in range(2):
                t = wts.tile([128, 256], F32R, name=f"wvt_{kc}")
                nc.sync.dma_start(t[:], wvt_d[kc])
                wvt[kc] = t
                t2 = wts.tile([128, 256], F32R, name=f"i256_{kc}")
                nc.sync.dma_start(t2[:], i256_d[kc])
                i256[kc] = t2
            bh = {}
            for mc in range(4):
                t = konst.tile([128, 1], F32, name=f"bh_{mc}")
                nc.sync.dma_start(t[:], bh_d[mc].unsqueeze(-1))
                bh[mc] = t
            b2c = {}
            for mc in range(2):
                t = konst.tile([128, 1], F32, name=f"b2_{mc}")
                nc.sync.dma_start(t[:], b2_d[mc].unsqueeze(-1))
                b2c[mc] = t
            mask = konst.tile([128, 9], F32, name="mask")
            nc.sync.dma_start(mask[:], mask_d[:])
            scv = konst.tile([128, 1], F32, name="scv")
            nc.sync.dma_start(scv[:], scale_d[:])

            zf32 = konst.tile([128, 512], F32, name="zf32")
            nc.gpsimd.memset(zf32[:], 0.0)
            vzero = konst.tile([128, 256], F32R, name="vzero")
            nc.vector.tensor_copy(vzero[:], zf32[:, :256])
            wpz = konst.tile([128, 384], F32R, name="wpz")
            nc.vector.tensor_copy(wpz[:], zf32[:, :384])

            # ---------- per-image pipeline ----------
            for img in [i % BS for i in range(BS * reps)]:
                # int8 staging (padded; border ring zeroed), then dequant to
                # bf16: padded xr for the conv, contiguous xu for -x lhsT
                xr, xu = [], []
                for cc in range(2):
                    q = xqp.tile([128, 66, 66], mybir.dt.int8, name="xq",
                                 tag="xq")
                    nc.gpsimd.memset(q[:, 0, :], 0)
                    nc.gpsimd.memset(q[:, 65, :], 0)
                    nc.gpsimd.memset(q[:, 1:65, 0], 0)
                    nc.gpsimd.memset(q[:, 1:65, 65], 0)
                    nc.sync.dma_start(q[:, 1:65, 1:65], xpad_d[img, cc])
                    t = data.tile([128, 66, 66], BF16, name="xr", tag="xr")
                    nc.scalar.activation(
                        t[:], q[:], mybir.ActivationFunctionType.Identity,
                        scale=scv[:])
                    xr.append(t)
                    u = xup.tile([128, HW], BF16, name="xu", tag="xu")
                    nc.scalar.activation(
                        u[:], q[:, 1:65, 1:65],
                        mybir.ActivationFunctionType.Identity, scale=scv[:])
                    xu.append(u)

                h_new = []
                for cc in range(2):
                    h_new.append(hnewp.tile([128, HW], F32R, name="h_new",
                                            tag="h_new"))

                # ---- ST1 fused conv+up1 -> GELU -> up2 -> residual
                for nt in range(NT):
                    r0 = 8 * nt
                    hid_sb = []
                    for mc in range(4):
                        hp = ps1.tile([128, 512], F32, space="PSUM",
                                      name="hid_ps", tag="hid_ps")
                        k = 0
                        for tau, (dy, dx) in enumerate(_TAUS):
                            for cc in range(2):
                                rhs = xr[cc][:, 1 + dy + r0:9 + dy + r0,
                                             1 + dx:65 + dx]
                                nc.tensor.matmul(
                                    hp[:], w1f[tau, cc, mc][:], rhs,
                                    start=(k == 0), stop=(k == 17))
                                k += 1
                        hs = hidp.tile([128, 512], F32R, name="hid_sb",
                                       tag="hid_sb")
                        nc.scalar.activation(hs[:], hp[:], GELU,
                                             bias=bh[mc][:])
                        hid_sb.append(hs)
                    for mc in range(2):
                        dp = ps2.tile([128, 512], F32, space="PSUM",
                                      name="dx_ps", tag="acc512")
                        for kc in range(4):
                            nc.tensor.matmul(dp[:], w2t[kc, mc][:],
                                             hid_sb[kc][:],
                                             start=(kc == 0), stop=False)
                        # + x via identity matmul (x is bf16; engines can't
                        # mix bf16 with f32-family operands)
                        nc.tensor.matmul(dp[:], ib128[:],
                                         xr[mc][:, 1 + r0:9 + r0, 1:65],
                                         start=False, stop=True)
                        # h_new = (dx + x) + b2
                        nc.scalar.activation(
                            h_new[mc][:, 512 * nt:512 * nt + 512], dp[:],
                            mybir.ActivationFunctionType.Identity,
                            bias=b2c[mc][:])

                if upto == "stage1":
                    for j in range(NCHUNK):
                        osb = small.tile([128, 256], mybir.dt.float8e4,
                                         name="osb", tag="osb")
                        nc.scalar.activation(
                            osb[:],
                            h_new[j % 2][:, min(128 * j, HW - 256):
                                         min(128 * j, HW - 256) + 256],
                            mybir.ActivationFunctionType.Copy)
                        nc.sync.dma_start(
                            _cap(out_d, (img * HW + 128 * j) * 256,
                                 [(256, 128), (1, 256)]), osb[:])
                    continue

                # ---- z = A @ h_new into padded flat layout
                z_pad = []
                for cc in range(2):
                    zt = zpadp.tile([128, ZP], F32R, name="z_pad",
                                    tag="z_pad")
                    # zero the pad zones (guard col + y=-1 row | y=64 row +
                    # guard): cols [0,65) and [ZP-65, ZP)
                    nc.scalar.activation(
                        zt[:, 0:65], zf32[:, 0:65],
                        mybir.ActivationFunctionType.Copy)
                    nc.scalar.activation(
                        zt[:, ZP - 65:ZP], zf32[:, 0:65],
                        mybir.ActivationFunctionType.Copy)
                    z_pad.append(zt)
                for nt in range(NT):
                    for mc in range(2):
                        zp = ps2.tile([128, 512], F32, space="PSUM",
                                      name="z_ps", tag="acc512")
                        for kc in range(2):
                            nc.tensor.matmul(
                                zp[:], at[kc, mc][:],
                                h_new[kc][:, 512 * nt:512 * nt + 512],
                                start=(kc == 0), stop=(kc == 1))
                        nc.vector.tensor_copy(
                            z_pad[mc][:, 65 + 512 * nt:65 + 512 * nt + 512],
                            zp[:])

                # ---- attention: per 128-pixel chunk
                v_sb = {}
                for k in range(NCHUNK + 1):
                    if k < NCHUNK:
                        # v[k] = (Wv h)^T via lhsT = h_new columns
                        vps = ps4.tile([128, 256], F32, space="PSUM",
                                       name="v_ps", tag="v_ps")
                        for kc in range(2):
                            nc.tensor.matmul(
                                vps[:], h_new[kc][:, 128 * k:128 * k + 128],
                                wvt[kc][:], start=(kc == 0), stop=(kc == 1))
                        vt = vpool.tile([128, 256], F32R, name="v_sb",
                                        tag="v_sb")
                        nc.vector.tensor_copy(vt[:], vps[:])
                        v_sb[k] = vt
                    if k < 1:
                        continue
                    j = k - 1
                    # Gram G = h^T z over the 258-wide band
                    gps = ps3.tile([128, 258], F32, space="PSUM",
                                   name="g_ps", tag="g_ps")
                    for kc in range(2):
                        nc.tensor.matmul(
                            gps[:], h_new[kc][:, 128 * j:128 * j + 128],
                            z_pad[kc][:, 128 * j:128 * j + 258],
                            start=(kc == 0), stop=(kc == 1))
                    gsb = small.tile([128, 258], F32, name="gsb", tag="gsb")
                    nc.scalar.activation(gsb[:], gps[:],
                                         mybir.ActivationFunctionType.Copy)
                    gd = gdram.tile([128, 258], F32, space="DRAM",
                                    name="g_dram", tag="g_dram")
                    nc.sync.dma_start(gd[:], gsb[:])
                    # diagonal extraction: s[p, (dy,dx)] = G[p, p+64(dy+1)+dx+1]
                    sc = small.tile([128, 9], F32, name="sc", tag="sc")
                    for a in range(3):
                        nc.sync.dma_start(
                            sc[:, 3 * a:3 * a + 3],
                            _cap(gd, gd.offset + 64 * a,
                                 [(259, 128), (1, 3)]))
                    # mask -> exp -> normalize(+mask numerator)
                    sm = small.tile([128, 9], F32, name="sm", tag="sm")
                    nc.vector.tensor_tensor(sm[:], sc[:], mask[:], op=MULT)
                    ex = small.tile([128, 9], F32, name="ex", tag="ex")
                    nc.scalar.activation(ex[:], sm[:], EXP)
                    sume = small.tile([128, 1], F32, name="sume", tag="sume")
                    nc.vector.tensor_reduce(sume[:], ex[:],
                                            axis=mybir.AxisListType.X, op=ADD)
                    rec = small.tile([128, 1], F32, name="rec", tag="rec")
                    nc.vector.reciprocal(rec[:], sume[:])
                    wn = small.tile([128, 9], F32R, name="wn", tag="wn")
                    nc.vector.scalar_tensor_tensor(
                        out=wn[:], in0=ex[:], scalar=rec[:], in1=mask[:],
                        op0=MULT, op1=MULT)
                    # scatter normalized weights into banded W' in DRAM
                    wp = wpdram.tile([384, 128], F32R, space="DRAM",
                                     name="wp_dram", tag="wp_dram")
                    nc.sync.dma_start(wp[:], wpz[:])  # zero background
                    for a in range(3):
                        nc.sync.dma_start(
                            _cap(wp, wp.offset + 8064 + 8192 * a,
                                 [(129, 128), (128, 3)]),
                            wn[:, 3 * a:3 * a + 3])
                    wl = []
                    for j3 in range(3):
                        wlt = wlp.tile([128, 128], F32R, name="wl", tag="wl")
                        nc.sync.dma_start(
                            wlt[:], wp[128 * j3:128 * j3 + 128, :])
                        wl.append(wlt)
                    # final = h^T (identity matmul) + W'^T v_band, one PSUM group
                    # delta = (h_new - x) + attn, shipped fp8 (host adds
                    # bf16(x) back: |delta| <= ~1 so fp8 stays in budget)
                    fp = ps5.tile([128, 256], F32, space="PSUM",
                                  name="fin_ps", tag="fin_ps")
                    for kc in range(2):
                        nc.tensor.matmul(
                            fp[:], h_new[kc][:, 128 * j:128 * j + 128],
                            i256[kc][:], start=(kc == 0), stop=False)
                        nc.tensor.matmul(
                            fp[:], xu[kc][:, 128 * j:128 * j + 128],
                            ni256[kc][:], start=False, stop=False)
                    for j3 in range(3):
                        kk = j - 1 + j3
                        vband = v_sb[kk][:] if 0 <= kk < NCHUNK else vzero[:]
                        nc.tensor.matmul(fp[:], wl[j3][:], vband,
                                         start=False, stop=(j3 == 2))
                    # delta chunk -> int8 (fixed scale DSTEP; |delta|<=~0.8
                    # so |int|<=~41 of 127) -> DRAM NHWC (pixel-major)
                    osb = small.tile([128, 256], mybir.dt.int8,
                                     name="osb", tag="osb")
                    nc.scalar.activation(osb[:], fp[:],
                                         mybir.ActivationFunctionType.Copy,
                                         scale=float(1.0 / DSTEP))
                    nc.sync.dma_start(
                        _cap(out_d, (img * HW + 128 * j) * 256,
                             [(256, 128), (1, 256)]),
                        osb[:])

    nc.compile()
    return nc


_NC_CACHE = {}


def _get_program(consts, step):
    key = hash((float(step),) + tuple((k, np.asarray(v).tobytes())
                                      for k, v in sorted(consts.items())))
    if _NC_CACHE.get("key") != key:
        _NC_CACHE["nc"] = _build_program(consts, step)
        _NC_CACHE["key"] = key
    return _NC_CACHE["nc"]


def _host_prepare(w_perc, b_perc, w_up1, b_up1, w_up2, b_up2, w_qkv, b_qkv):
    w_perc = np.asarray(w_perc, np.float32)
    b_perc = np.asarray(b_perc, np.float32)
    w_up1 = np.asarray(w_up1, np.float32)
    b_up1 = np.asarray(b_up1, np.float32)
    w_up2 = np.asarray(w_up2, np.float32)
    b_up2 = np.asarray(b_up2, np.float32)
    w_qkv = np.asarray(w_qkv, np.float32)
    b_qkv = np.asarray(b_qkv, np.float32)
    assert np.allclose(b_qkv, 0.0), "kernel assumes zero qkv bias (A-trick)"

    wp = w_perc[:, 0]                       # [3C, 3, 3]
    W1 = w_up1[:, :, 0, 0]                  # [2C, 3C]
    W1r = W1.reshape(C2, C, 3)              # [d, g, t]
    wpr = wp.reshape(C, 3, 3, 3)            # [g, t, dy, dx]
    W1f = np.einsum("dgt,gtyx->yxdg", W1r, wpr).reshape(9, C2, C)
    bh = b_up1 + W1 @ b_perc                # [2C]
    W2 = w_up2[:, :, 0, 0]                  # [C, 2C]
    Wq, Wk, Wv = w_qkv[:C], w_qkv[C:C2], w_qkv[C2:]
    A = (Wq.T @ Wk) / math.sqrt(C)          # [C, C]

    w1f_t = np.empty((9, 2, 4, 128, 128), np.float32)
    for tau in range(9):
        for cc in range(2):
            for mc in range(4):
                w1f_t[tau, cc, mc] = W1f[tau][mc * 128:(mc + 1) * 128,
                                             cc * 128:(cc + 1) * 128].T
    w2t_t = np.empty((4, 2, 128, 128), np.float32)
    for kc in range(4):
        for mc in range(2):
            w2t_t[kc, mc] = W2[mc * 128:(mc + 1) * 128,
                               kc * 128:(kc + 1) * 128].T
    at_t = np.empty((2, 2, 128, 128), np.float32)
    for kc in range(2):
        for mc in range(2):
            at_t[kc, mc] = A[mc * 128:(mc + 1) * 128,
                             kc * 128:(kc + 1) * 128].T
    wvt_t = np.ascontiguousarray(Wv.T.reshape(2, 128, 256))
    i256_t = np.ascontiguousarray(np.eye(256, dtype=np.float32)
                                  .reshape(2, 128, 256))
    bh_t = np.ascontiguousarray(bh.reshape(4, 128))
    b2_t = np.ascontiguousarray(b_up2.reshape(2, 128))

    maskt = np.ones((128, 9), np.float32)
    for p in range(128):
        xx = p % 64
        for dy in (-1, 0, 1):
            for dx in (-1, 0, 1):
                if (xx == 0 and dx == -1) or (xx == 63 and dx == 1):
                    maskt[p, (dy + 1) * 3 + (dx + 1)] = 0.0

    return dict(w1f=w1f_t, w2t=w2t_t, at=at_t, wvt=wvt_t, i256=i256_t,
                bh=bh_t, b2=b2_t, mask=maskt)


def _quant_step(h):
    return np.float32(max(np.abs(h).max() / 127.0, 1e-20))


def _make_in_maps(h):
    """Per-core input maps: NCHW int8 images (scale is baked in the NEFF)."""
    h = np.asarray(h, np.float32)
    step = _quant_step(h)
    hq = np.clip(np.round(h / step), -127, 127).astype(np.int8)
    in_maps = []
    for core in range(NCORES):
        hx = hq[core * BS:(core + 1) * BS].transpose(0, 3, 1, 2)  # [BS,C,H,W]
        in_maps.append(
            {"xpad": np.ascontiguousarray(hx.reshape(BS, 2, 128, 64, 64))})
    return in_maps


_RUN_CACHE = {}


def _build_runner(nc):
    """Lean SPMD dispatch: mirrors bass2jax.run_bass_via_pjrt, but the jitted
    executable is cached across calls and the output placeholder operand is a
    persistent (non-donated) device array — so per dispatch only xpad crosses
    H2D and out crosses D2H. Valid because this kernel writes every output
    element (PJRT custom-call results are allocated uninitialized)."""
    import jax
    import jax.numpy as jnp
    from jax.sharding import Mesh, NamedSharding, PartitionSpec
    from jax.experimental.shard_map import shard_map

    from concourse import bass2jax
    from concourse.bass2jax import _bass_exec_p, install_neuronx_cc_hook

    install_neuronx_cc_hook()

    partition_name = (nc.partition_id_tensor.name
                      if nc.partition_id_tensor else None)
    in_names, out_names, out_avals = [], [], []
    for alloc in nc.m.functions[0].allocations:
        if not isinstance(alloc, mybir.MemoryLocationSet):
            continue
        name = alloc.memorylocations[0].name
        if alloc.kind == "ExternalInput":
            if name != partition_name:
                in_names.append(name)
        elif alloc.kind == "ExternalOutput":
            out_names.append(name)
            out_avals.append(jax.core.ShapedArray(
                tuple(alloc.tensor_shape), mybir.dt.np(alloc.dtype)))
    n_params = len(in_names)
    in_names = in_names + out_names
    if partition_name is not None:
        in_names.append(partition_name)

    def _body(*args):
        operands = list(args)
        if partition_name is not None:
            operands.append(bass2jax.partition_id_tensor())
        outs = _bass_exec_p.bind(
            *operands,
            out_avals=tuple(out_avals),
            in_names=tuple(in_names),
            out_names=tuple(out_names),
            lowering_input_output_aliases=(),
            sim_require_finite=True,
            sim_require_nnan=True,
            nc=nc,
        )
        return tuple(outs)

    devices = jax.devices()[:NCORES]
    mesh = Mesh(np.asarray(devices), ("core",))
    nio = n_params + len(out_names)
    jitted = jax.jit(
        shard_map(_body, mesh=mesh,
                  in_specs=(PartitionSpec("core"),) * nio,
                  out_specs=(PartitionSpec("core"),) * len(out_names),
                  check_rep=False),
        keep_unused=True,
    )
    in_shapes = []
    for alloc in nc.m.functions[0].allocations:
        if not isinstance(alloc, mybir.MemoryLocationSet):
            continue
        name = alloc.memorylocations[0].name
        if alloc.kind == "ExternalInput" and name in in_names[:n_params]:
            in_shapes.append((name, tuple(alloc.tensor_shape),
                              mybir.dt.np(alloc.dtype)))
    in_shapes.sort(key=lambda t: in_names.index(t[0]))
    arg_structs = [
        jax.ShapeDtypeStruct((NCORES * s[0], *s[1:]), dt)
        for _, s, dt in in_shapes
    ] + [
        jax.ShapeDtypeStruct((NCORES * a.shape[0], *a.shape[1:]), a.dtype)
        for a in out_avals
    ]
    try:
        from concourse.bass2jax import fast_dispatch_compile
        fn = fast_dispatch_compile(
            lambda: jitted.lower(*arg_structs).compile())
    except Exception:
        fn = jitted
    sh = NamedSharding(mesh, PartitionSpec("core"))
    placeholders = [
        jax.device_put(np.zeros((NCORES * a.shape[0], *a.shape[1:]), a.dtype),
                       sh)
        for a in out_avals
    ]
    return dict(fn=fn, placeholders=placeholders,
                in_names=in_names[:n_params], out_names=out_names)


def _dispatch(nc, in_maps):
    r = _RUN_CACHE.get("runner")
    if r is None or _RUN_CACHE.get("nc") is not nc:
        r = _build_runner(nc)
        _RUN_CACHE["runner"] = r
        _RUN_CACHE["nc"] = nc
    concat_in = [
        np.concatenate([m[name] for m in in_maps], axis=0)
        for name in r["in_names"]
    ]
    outs = r["fn"](*concat_in, *r["placeholders"])
    return [np.asarray(o) for o in outs]


def kernel(h, w_perc, b_perc, w_up1, b_up1, w_up2, b_up2, w_qkv, b_qkv):
    consts = _host_prepare(w_perc, b_perc, w_up1, b_up1, w_up2, b_up2,
                           w_qkv, b_qkv)
    nc = _get_program(consts, _quant_step(np.asarray(h, np.float32)))
    in_maps = _make_in_maps(h)
    delta = _dispatch(nc, in_maps)[0]     # [B, 64, 64, 256] int8 * DSTEP
    # add back the TRUE f32 x (not the quantized x-hat): the direct input
    # quantization error then cancels; only propagated effects remain
    return (np.asarray(h, np.float32)
            + delta.astype(np.float32) * np.float32(DSTEP))
